# revision 6
# baseline (speedup 1.0000x reference)
"""Trainium2 Bass kernel for nn_CaC_50637664420271.

Computes, for x:[16,256,64,64]:
  feat_k = wk @ x + bk  (1x1 conv), feat_q = wq @ x + bq
  krnl[n,c,3,3] = bmm(feat_k, feat_q^T)  -> BatchNorm (train stats) ->
  out = mean_d sigmoid(depthwise_conv(x, krnl, dilation=d)), d in {1,2,3}

Sharding: pure data-parallel over batch (2 samples / core, 8 cores), with a
tiny AllReduce of per-channel (sum, sumsq) of krnl for the BN batch stats.

All heavy matmuls run in fp32r mode (full PE rate, ~1e-4 rounding): the
1x1-conv features, the per-sample kernel bmm, and most depthwise-conv taps
(diag-weight matmuls over shifted windows of the zero-padded image,
accumulated in PSUM). The center tap rides the DVE as a fused
scalar_tensor_tensor that also evacuates PSUM into the SBUF z-buffer;
a few more taps run as DVE STT / GPSIMD mul+add. Sigmoids (ScalarE) emit
bf16; the three dilations merge via identity bf16 matmuls accumulating in
PSUM, evacuated by ScalarE with a fused 1/3 scale.
"""
import os
import numpy as np
import ml_dtypes

import concourse.bass as bass
import concourse.bacc as bacc
import concourse.tile as tile
import concourse.mybir as mybir
from concourse import bass_utils

N_CORES = 8
NLOC = 2            # samples per core
C = 256
H = W = 64
HW = H * W          # 4096
S = 3
PAD = 3
WP = W + 2 * PAD    # padded row width 70
HP = H + 2 * PAD
PSZ = WP * HP       # 70*70 = 4900 padded image size
CB = C // 128       # channel blocks per sample (2)
NU = NLOC * CB      # units per core (4)
FQ = S * S          # 9
TQ = 10             # padded tap columns (even for fp32r)
FKQ = C + TQ        # 266 fused feature columns (col 265 dummy zero)
BN_EPS = 1e-5
BN_CNT = 16 * FQ    # 144 elements per channel in BN stats

CH_ROWS = 16                      # conv psum chunk = 16 rows = 1024 px
NCHUNK = H // CH_ROWS             # 4
CSPAN = CH_ROWS * W               # 1024
MG = 512                          # merge chunk columns

# per-dilation engine split of the 9 taps
TAPS = {
    1: {"pe": (0, 1, 2, 6, 8), "dve": (3, 5), "gp": (7,)},
    2: {"pe": (0, 1, 2, 3, 5, 6), "dve": (7, 8), "gp": ()},
    3: {"pe": (0, 1, 2, 3, 5, 6), "dve": (8,), "gp": (7,)},
}
DIAG_TAPS = sorted({t for c in TAPS.values() for t in c["pe"]})

dt = mybir.dt.float32
dr = mybir.dt.float32r
db = mybir.dt.bfloat16
ALU = mybir.AluOpType
AF = mybir.ActivationFunctionType
AX = mybir.AxisListType


def tap_dydx(t, d):
    return d * (t // S - 1), d * (t % S - 1)


def _body(nc, tc, tens):
    x_d, w_d, bk_d, bq_d, g_d, b_d, out_d = tens
    with tc.tile_pool(name="const", bufs=1) as cpool, \
         tc.tile_pool(name="pimg", bufs=2) as ppool, \
         tc.tile_pool(name="big", bufs=4) as bpool, \
         tc.tile_pool(name="fb", bufs=4) as fpool, \
         tc.tile_pool(name="sig", bufs=4) as spool, \
         tc.tile_pool(name="ga", bufs=1) as apool, \
         tc.tile_pool(name="diag", bufs=8) as gpool, \
         tc.tile_pool(name="small", bufs=1) as vpool, \
         tc.tile_pool(name="work", bufs=4) as wpool, \
         tc.tile_pool(name="ps", bufs=2, space="PSUM") as psA, \
         tc.tile_pool(name="mg", bufs=2, space="PSUM") as psB, \
         tc.tile_pool(name="kp", bufs=1, space="PSUM") as psK, \
         tc.tile_pool(name="dram", bufs=2, space="DRAM") as dpool:

        # ---- constants / weights ----------------------------------------
        ident_d = nc.inline_tensor(np.eye(128, dtype=np.float32),
                                   name="ident")
        ident = cpool.tile([128, 128], dt, tag="ident")
        nc.sync.dma_start(ident[:], ident_d.ap())
        identb_d = nc.inline_tensor(np.eye(128, dtype=ml_dtypes.bfloat16),
                                    name="identb")
        identb = cpool.tile([128, 128], db, tag="identb")
        nc.sync.dma_start(identb[:], identb_d.ap())

        wkq = []
        for ki in range(CB):
            t = cpool.tile([128, FKQ], dr, tag=f"wkq{ki}", name=f"wkq{ki}")
            nc.sync.dma_start(t[:], w_d.ap()[ki * 128:(ki + 1) * 128, :])
            wkq.append(t)

        ones_r = cpool.tile([1, 128], dt, tag="ones")
        nc.vector.memset(ones_r[:], 1.0)
        bias_r = cpool.tile([1, FKQ], dt, tag="biasr")
        nc.vector.memset(bias_r[:, C + FQ:FKQ], 0.0)
        nc.sync.dma_start(bias_r[0:1, 0:C],
                          bk_d.ap().rearrange("(p f) -> p f", p=1))
        nc.sync.dma_start(bias_r[0:1, C:C + FQ],
                          bq_d.ap().rearrange("(p f) -> p f", p=1))

        gam, bet = [], []
        for cb in range(CB):
            gt = cpool.tile([128, 1], dt, tag=f"g{cb}", name=f"g{cb}")
            bt = cpool.tile([128, 1], dt, tag=f"b{cb}", name=f"b{cb}")
            nc.sync.dma_start(
                gt[:], g_d.ap().rearrange("(p f) -> p f", f=1)[
                    cb * 128:(cb + 1) * 128, :])
            nc.sync.dma_start(
                bt[:], b_d.ap().rearrange("(p f) -> p f", f=1)[
                    cb * 128:(cb + 1) * 128, :])
            gam.append(gt)
            bet.append(bt)

        def pwin(pt, r0, nr, dy, dx):
            g = pt[:].rearrange("p (r c) -> p r c", c=WP)
            return g[:, PAD + r0 + dy:PAD + r0 + dy + nr,
                     PAD + dx:PAD + dx + W]

        # broadcast bias row to all 128 partitions via PE
        bb_ps = psB.tile([128, FKQ], dt, tag="mg", name="bbps")
        nc.tensor.matmul(bb_ps[:], ones_r[:], bias_r[:], start=True,
                         stop=True)
        bias_bc = cpool.tile([128, FKQ], dt, tag="biasbc")
        nc.vector.tensor_copy(bias_bc[:], bb_ps[:])

        # ---- features + per-sample kernel bmm ---------------------------
        # fkT[p,c'] = sum_c x[c,p] wkq^T[c,c'] + bias  (pixel-major)
        # krnl[c,t] = sum_p fkT[p,c] * fqT[p,t]
        krnl = [[vpool.tile([128, TQ], dt, tag=f"krnl{s}{cb}",
                            name=f"krnl{s}{cb}")
                 for cb in range(CB)] for s in range(NLOC)]
        for s in range(NLOC):
            xc = []
            for cb in range(CB):
                t = bpool.tile([128, HW], dr, tag="big", name=f"xc{s}{cb}")
                for q in range(2):
                    nc.sync.dma_start(
                        t[:, q * (HW // 2):(q + 1) * (HW // 2)],
                        x_d.ap()[s, cb * 128:(cb + 1) * 128].rearrange(
                            "p r c -> p (r c)")[:, q * (HW // 2):
                                                (q + 1) * (HW // 2)])
                xc.append(t)
            kps = [psK.tile([128, TQ], dt, tag=f"kp{cb}", name=f"kp{s}{cb}")
                   for cb in range(CB)]
            for pb2 in range(HW // 256):     # two 128-px blocks per iter
                # h-th block at psum offset 512*h to keep each matmul dst
                # within one 2KB PSUM bank
                fp = psA.tile([128, 1024], dt, tag="ps", name="feat")
                for h in range(2):
                    pb = 2 * pb2 + h
                    for ki in range(CB):
                        nc.tensor.matmul(
                            fp[:, h * 512:h * 512 + FKQ],
                            xc[ki][:, pb * 128:(pb + 1) * 128],
                            wkq[ki][:], start=(ki == 0), stop=(ki == CB - 1))
                fb = fpool.tile([128, 2 * FKQ], dr, tag="fb", name="fb")
                for h in range(2):
                    nc.vector.tensor_tensor(
                        out=fb[:, h * FKQ:(h + 1) * FKQ],
                        in0=fp[:, h * 512:h * 512 + FKQ],
                        in1=bias_bc[:], op=ALU.add)
                for cb in range(CB):
                    for h in range(2):
                        nc.tensor.matmul(
                            kps[cb][:],
                            fb[:, h * FKQ + cb * 128:h * FKQ + (cb + 1) * 128],
                            fb[:, h * FKQ + C:h * FKQ + C + TQ],
                            start=(pb2 == 0 and h == 0),
                            stop=(pb2 == HW // 256 - 1 and h == 1))
            for cb in range(CB):
                nc.vector.tensor_copy(krnl[s][cb][:], kps[cb][:])

        # ---- BN stats + AllReduce ---------------------------------------
        loc = []
        for cb in range(CB):
            st = vpool.tile([128, 2], dt, tag=f"st{cb}", name=f"st{cb}")
            tmp = wpool.tile([128, TQ], dt, tag="sq", name="sq")
            prt = wpool.tile([128, 4], dt, tag="prt", name="prt")
            for s in range(NLOC):
                nc.vector.tensor_reduce(prt[:, s:s + 1], krnl[s][cb][:],
                                        AX.X, ALU.add)
                nc.vector.tensor_tensor(out=tmp[:], in0=krnl[s][cb][:],
                                        in1=krnl[s][cb][:], op=ALU.mult)
                nc.vector.tensor_reduce(prt[:, 2 + s:3 + s], tmp[:],
                                        AX.X, ALU.add)
            nc.vector.tensor_tensor(out=st[:, 0:1], in0=prt[:, 0:1],
                                    in1=prt[:, 1:2], op=ALU.add)
            nc.vector.tensor_tensor(out=st[:, 1:2], in0=prt[:, 2:3],
                                    in1=prt[:, 3:4], op=ALU.add)
            loc.append(st)

        ib = dpool.tile([CB, 128, 2], dt)
        ob = dpool.tile([CB, 128, 2], dt)
        for cb in range(CB):
            nc.gpsimd.dma_start(ib[cb], loc[cb][:])
        if os.environ.get("PROF_NO_CC"):
            nc.gpsimd.dma_start(ob[:], ib[:])
        else:
            nc.gpsimd.collective_compute(
                "AllReduce", ALU.add, replica_groups=[list(range(N_CORES))],
                ins=[ib.opt()], outs=[ob.opt()])

        eps_t = vpool.tile([128, 1], dt, tag="eps")
        nc.vector.memset(eps_t[:], BN_EPS)
        scale, shift = [], []
        for cb in range(CB):
            gl = vpool.tile([128, 2], dt, tag=f"gl{cb}", name=f"gl{cb}")
            nc.gpsimd.dma_start(gl[:], ob[cb])
            mean = wpool.tile([128, 1], dt, tag="mean", name="mean")
            sc = vpool.tile([128, 1], dt, tag=f"sc{cb}", name=f"sc{cb}")
            sh = vpool.tile([128, 1], dt, tag=f"sh{cb}", name=f"sh{cb}")
            t0 = wpool.tile([128, 1], dt, tag="bn0", name="bn0")
            t1 = wpool.tile([128, 1], dt, tag="bn1", name="bn1")
            nc.vector.tensor_scalar_mul(mean[:], gl[:, 0:1], 1.0 / BN_CNT)
            nc.vector.tensor_tensor(out=t0[:], in0=mean[:], in1=mean[:],
                                    op=ALU.mult)
            nc.vector.scalar_tensor_tensor(
                out=t1[:], in0=gl[:, 1:2], scalar=1.0 / BN_CNT, in1=t0[:],
                op0=ALU.mult, op1=ALU.subtract)
            nc.scalar.activation(t0[:], t1[:], AF.Sqrt, bias=eps_t[:])
            nc.vector.reciprocal(t1[:], t0[:])
            nc.vector.tensor_tensor(out=sc[:], in0=gam[cb][:], in1=t1[:],
                                    op=ALU.mult)
            nc.vector.tensor_tensor(out=t0[:], in0=mean[:], in1=sc[:],
                                    op=ALU.mult)
            nc.vector.tensor_tensor(out=sh[:], in0=bet[cb][:], in1=t0[:],
                                    op=ALU.subtract)
            scale.append(sc)
            shift.append(sh)

        # normalized per-tap weights: w = krnl*scale + shift
        wnorm = []
        for s in range(NLOC):
            wr = []
            for cb in range(CB):
                wn = vpool.tile([128, TQ], dt, tag=f"wn{s}{cb}",
                                name=f"wn{s}{cb}")
                nc.vector.tensor_scalar(
                    out=wn[:], in0=krnl[s][cb][:],
                    scalar1=scale[cb][:], scalar2=shift[cb][:],
                    op0=ALU.mult, op1=ALU.add)
                wr.append(wn)
            wnorm.append(wr)

        # ---- depthwise convs + sigmoid + merge --------------------------
        for u in range(NU):
            s, cb = divmod(u, CB)
            wn = wnorm[s][cb]
            # zero-padded image (fp32r; DMA rounds on load)
            pt = ppool.tile([128, PSZ], dr, tag="pimg", name=f"pc{u}")
            pg = pt[:].rearrange("p (r c) -> p r c", c=WP)
            nc.gpsimd.memset(pt[:, 0:PAD * WP + PAD].bitcast(dt), 0.0)
            nc.gpsimd.memset(pt[:, PSZ - PAD * WP - PAD:PSZ].bitcast(dt), 0.0)
            nc.gpsimd.memset(pg[:, PAD:PAD + H, 0:PAD].bitcast(dt), 0.0)
            nc.gpsimd.memset(pg[:, PAD:PAD + H, PAD + W:WP].bitcast(dt), 0.0)
            for q in range(2):
                rr = H // 2
                nc.sync.dma_start(
                    pg[:, PAD + q * rr:PAD + (q + 1) * rr, PAD:PAD + W],
                    x_d.ap()[s, cb * 128:(cb + 1) * 128,
                             q * rr:(q + 1) * rr])
            diag = {}
            for t in DIAG_TAPS:
                dg = gpool.tile([128, 128], dr, tag="diag", name=f"dg{u}_{t}")
                nc.gpsimd.tensor_scalar_mul(dg[:], ident[:], wn[:, t:t + 1])
                diag[t] = dg
            sgs = []
            for di, d in enumerate((1, 2, 3)):
                cfg = TAPS[d]
                zb = bpool.tile([128, HW], dt, tag="big", name=f"zb{u}_{di}")
                zb3 = zb[:].rearrange("p (r c) -> p r c", c=W)
                for ci in range(NCHUNK):
                    r0 = ci * CH_ROWS
                    pz = psA.tile([128, CSPAN], dt, tag="ps", name=f"pz{di}")
                    for i, t in enumerate(cfg["pe"]):
                        dy, dx = tap_dydx(t, d)
                        for sp in range(0, CSPAN, MG):
                            nc.tensor.matmul(
                                pz[:, sp:sp + MG], diag[t][:],
                                pwin(pt, r0 + sp // W, MG // W, dy, dx),
                                start=(i == 0),
                                stop=(i == len(cfg["pe"]) - 1))
                    # fused center tap + psum evacuation
                    nc.vector.scalar_tensor_tensor(
                        out=zb3[:, r0:r0 + CH_ROWS, :],
                        in0=pwin(pt, r0, CH_ROWS, 0, 0),
                        scalar=wn[:, 4:5],
                        in1=pz[:].rearrange("p (r c) -> p r c", c=W),
                        op0=ALU.mult, op1=ALU.add)
                for t in cfg["dve"]:
                    dy, dx = tap_dydx(t, d)
                    nc.vector.scalar_tensor_tensor(
                        out=zb3, in0=pwin(pt, 0, H, dy, dx),
                        scalar=wn[:, t:t + 1], in1=zb3,
                        op0=ALU.mult, op1=ALU.add)
                for t in cfg["gp"]:
                    dy, dx = tap_dydx(t, d)
                    ga = apool.tile([128, HW], dt, tag="ga",
                                    name=f"ga{u}_{di}")
                    nc.gpsimd.tensor_scalar_mul(
                        ga[:].rearrange("p (r c) -> p r c", c=W),
                        pwin(pt, 0, H, dy, dx), wn[:, t:t + 1])
                    nc.gpsimd.tensor_tensor(out=zb[:], in0=zb[:], in1=ga[:],
                                            op=ALU.add)
                sg = spool.tile([128, HW], db, tag="sig", name=f"sg{u}_{di}")
                nc.scalar.activation(sg[:], zb[:], AF.Sigmoid)
                sgs.append(sg)
            # merge the three dilations: psum += I * sg_d, evac with 1/3
            ext = bpool.tile([128, HW], dt, tag="big", name=f"ext{u}")
            for off in range(0, HW, MG):
                pm = psB.tile([128, MG], dt, tag="mg", name=f"pm{u}")
                for di in range(3):
                    nc.tensor.matmul(pm[:], identb[:],
                                     sgs[di][:, off:off + MG],
                                     start=(di == 0), stop=(di == 2))
                nc.scalar.activation(ext[:, off:off + MG], pm[:], AF.Copy,
                                     scale=1.0 / 3.0)
            for q in range(2):
                nc.sync.dma_start(
                    out_d.ap()[s, cb * 128:(cb + 1) * 128,
                               q * 32:(q + 1) * 32],
                    ext[:].rearrange("p (r c) -> p r c", c=W)[
                        :, q * 32:(q + 1) * 32, :])


def _build():
    nc = bacc.Bacc("TRN2", debug=False, num_devices=N_CORES,
                   target_bir_lowering=False)
    x_d = nc.dram_tensor("x", [NLOC, C, H, W], dr, kind="ExternalInput")
    w_d = nc.dram_tensor("wkqt", [C, FKQ], dr, kind="ExternalInput")
    bk_d = nc.dram_tensor("bk", [C], dt, kind="ExternalInput")
    bq_d = nc.dram_tensor("bq", [FQ], dt, kind="ExternalInput")
    g_d = nc.dram_tensor("gamma", [C], dt, kind="ExternalInput")
    b_d = nc.dram_tensor("beta", [C], dt, kind="ExternalInput")
    out_d = nc.dram_tensor("out", [NLOC, C, H, W], dt, kind="ExternalOutput")
    with tile.TileContext(nc) as tc:
        _body(nc, tc, (x_d, w_d, bk_d, bq_d, g_d, b_d, out_d))
    nc.compile()
    return nc


_nc_cache = None
last_results = None


def kernel(x, wk, bk, wq, bq, gamma, beta):
    global _nc_cache, last_results
    if _nc_cache is None:
        _nc_cache = _build()
    nc = _nc_cache
    x = np.ascontiguousarray(x, dtype=np.float32)
    wkqt = np.concatenate(
        [np.asarray(wk, np.float32).T, np.asarray(wq, np.float32).T,
         np.zeros((C, TQ - FQ), np.float32)], axis=1)  # [C, 266]
    wkqt = np.ascontiguousarray(wkqt)
    in_maps = []
    for c in range(N_CORES):
        sl = slice(c * NLOC, (c + 1) * NLOC)
        in_maps.append({
            "x": x[sl],
            "wkqt": wkqt,
            "bk": np.ascontiguousarray(bk, np.float32),
            "bq": np.ascontiguousarray(bq, np.float32),
            "gamma": np.ascontiguousarray(gamma, np.float32),
            "beta": np.ascontiguousarray(beta, np.float32),
        })
    res = bass_utils.run_bass_kernel_spmd(
        nc, in_maps, core_ids=list(range(N_CORES)))
    last_results = res
    out = np.concatenate([res.results[c]["out"] for c in range(N_CORES)],
                         axis=0)
    return out


# revision 18
# speedup vs baseline: 1.2468x; 1.2468x over previous
"""Trainium2 Bass kernel for nn_CaC_50637664420271.

Computes, for x:[16,256,64,64]:
  feat_k = wk @ x + bk  (1x1 conv), feat_q = wq @ x + bq
  krnl[n,c,3,3] = bmm(feat_k, feat_q^T)  -> BatchNorm (train stats) ->
  out = mean_d sigmoid(depthwise_conv(x, krnl, dilation=d)), d in {1,2,3}

Sharding: pure data-parallel over batch (2 samples / core, 8 cores), with a
tiny AllReduce of per-channel (sum, sumsq) of krnl for the BN batch stats.

All heavy matmuls run in fp32r mode (full PE rate, ~1e-4 rounding): the
1x1-conv features, the per-sample kernel bmm, and most depthwise-conv taps
(diag-weight matmuls over shifted windows of the zero-padded image,
accumulated in PSUM). The center tap rides the DVE as a fused
scalar_tensor_tensor that also evacuates PSUM into the SBUF z-buffer;
a few more taps run as DVE STT / GPSIMD mul+add. Sigmoids (ScalarE) emit
bf16; the three dilations merge via identity bf16 matmuls accumulating in
PSUM, evacuated by ScalarE with a fused 1/3 scale.
"""
import os
import numpy as np
import ml_dtypes

import concourse.bass as bass
import concourse.bacc as bacc
import concourse.tile as tile
import concourse.mybir as mybir
from concourse import bass_utils

N_CORES = 8
NLOC = 2            # samples per core
C = 256
H = W = 64
HW = H * W          # 4096
S = 3
PAD = 3
WP = W + 2 * PAD    # padded row width 70
HP = H + 2 * PAD
PSZ = WP * HP       # 70*70 = 4900 padded image size
CB = C // 128       # channel blocks per sample (2)
NU = NLOC * CB      # units per core (4)
FQ = S * S          # 9
TQ = 10             # padded tap columns (even for fp32r)
FKQ = C + TQ        # 266 fused feature columns (col 265 dummy zero)
BN_EPS = 1e-5
BN_CNT = 16 * FQ    # 144 elements per channel in BN stats

CH_ROWS = 8                       # conv psum chunk = 8 rows = 512 px (1 bank)
NCHUNK = H // CH_ROWS             # 8
CSPAN = CH_ROWS * W               # 512
MG = 512                          # merge chunk columns

# per-dilation engine split of the 9 taps; last unit avoids Pool taps to
# shorten the drain-out critical path
TAPS = {
    1: {"pe": (0, 1, 2, 6, 8), "dve": (3, 5), "gp": (7,)},
    2: {"pe": (0, 1, 2, 3, 5, 6, 8), "dve": (7,), "gp": ()},
    3: {"pe": (0, 1, 2, 3, 5, 6), "dve": (8,), "gp": (7,)},
}
TAPS_LAST = {
    1: {"pe": (0, 1, 2, 6, 7, 8), "dve": (3, 5), "gp": ()},
    2: {"pe": (0, 1, 2, 3, 5, 6, 8), "dve": (7,), "gp": ()},
    3: {"pe": (0, 1, 2, 3, 5, 6, 7), "dve": (8,), "gp": ()},
}
DIAG_TAPS = sorted({t for cc in (TAPS, TAPS_LAST)
                    for c in cc.values() for t in c["pe"]})

dt = mybir.dt.float32
dr = mybir.dt.float32r
db = mybir.dt.bfloat16
ALU = mybir.AluOpType
AF = mybir.ActivationFunctionType
AX = mybir.AxisListType


def tap_dydx(t, d):
    return d * (t // S - 1), d * (t % S - 1)


def _body(nc, tc, tens):
    x_d, w_d, bk_d, bq_d, g_d, b_d, out_d = tens
    with tc.tile_pool(name="const", bufs=1) as cpool, \
         tc.tile_pool(name="pimg", bufs=2) as ppool, \
         tc.tile_pool(name="big", bufs=6) as bpool, \
         tc.tile_pool(name="fb", bufs=4) as fpool, \
         tc.tile_pool(name="sig", bufs=6) as spool, \
         tc.tile_pool(name="ga", bufs=1) as apool, \
         tc.tile_pool(name="diag", bufs=28) as gpool, \
         tc.tile_pool(name="small", bufs=1) as vpool, \
         tc.tile_pool(name="work", bufs=4) as wpool, \
         tc.tile_pool(name="ps", bufs=6, space="PSUM") as psA, \
         tc.tile_pool(name="mg", bufs=2, space="PSUM") as psB, \
         tc.tile_pool(name="dram", bufs=2, space="DRAM") as dpool:

        # ---- constants / weights ----------------------------------------
        ident_d = nc.inline_tensor(np.eye(128, dtype=np.float32),
                                   name="ident")
        ident = cpool.tile([128, 128], dt, tag="ident")
        nc.sync.dma_start(ident[:], ident_d.ap())
        identb_d = nc.inline_tensor(np.eye(128, dtype=ml_dtypes.bfloat16),
                                    name="identb")
        identb = cpool.tile([128, 128], db, tag="identb")
        nc.sync.dma_start(identb[:], identb_d.ap())

        wkq = []
        for ki in range(CB):
            t = cpool.tile([128, FKQ], db, tag=f"wkq{ki}", name=f"wkq{ki}")
            nc.sync.dma_start(t[:], w_d.ap()[ki * 128:(ki + 1) * 128, :])
            wkq.append(t)

        ones_r = cpool.tile([1, 128], db, tag="ones")
        nc.vector.memset(ones_r[:], 1.0)
        bias_f = cpool.tile([1, FKQ], dt, tag="biasf")
        nc.vector.memset(bias_f[:, C + FQ:FKQ], 0.0)
        nc.sync.dma_start(bias_f[0:1, 0:C],
                          bk_d.ap().rearrange("(p f) -> p f", p=1))
        nc.sync.dma_start(bias_f[0:1, C:C + FQ],
                          bq_d.ap().rearrange("(p f) -> p f", p=1))
        bias_r = cpool.tile([1, FKQ], db, tag="biasr")
        nc.vector.tensor_copy(bias_r[:], bias_f[:])

        gam, bet = [], []
        for cb in range(CB):
            gt = cpool.tile([128, 1], dt, tag=f"g{cb}", name=f"g{cb}")
            bt = cpool.tile([128, 1], dt, tag=f"b{cb}", name=f"b{cb}")
            nc.sync.dma_start(
                gt[:], g_d.ap().rearrange("(p f) -> p f", f=1)[
                    cb * 128:(cb + 1) * 128, :])
            nc.sync.dma_start(
                bt[:], b_d.ap().rearrange("(p f) -> p f", f=1)[
                    cb * 128:(cb + 1) * 128, :])
            gam.append(gt)
            bet.append(bt)

        def pwin(pt, r0, nr, dy, dx):
            g = pt[:].rearrange("p (r c) -> p r c", c=WP)
            return g[:, PAD + r0 + dy:PAD + r0 + dy + nr,
                     PAD + dx:PAD + dx + W]

        # ---- features + per-sample kernel bmm ---------------------------
        # fkT[p,c'] = sum_c x[c,p] wkq^T[c,c'] + bias  (pixel-major)
        # krnl[c,t] = sum_p fkT[p,c] * fqT[p,t]
        krnl = [[vpool.tile([128, TQ], dt, tag=f"krnl{s}{cb}",
                            name=f"krnl{s}{cb}")
                 for cb in range(CB)] for s in range(NLOC)]
        for s in range(NLOC):
            xc = []
            for cb in range(CB):
                t = bpool.tile([128, HW], db, tag="xcb", name=f"xc{s}{cb}")
                eng = nc.sync if cb == 0 else nc.scalar
                eng.dma_start(
                    t[:], x_d.ap()[s, cb * 128:(cb + 1) * 128].rearrange(
                        "p r c -> p (r c)"))
                xc.append(t)
            kpt = psB.tile([128, 512], dt, tag="mg", name=f"kp{s}")
            kps = [kpt[:, cb * 256:cb * 256 + TQ] for cb in range(CB)]
            for pb in range(HW // 128):
                fp = psA.tile([128, FKQ], dt, tag="ps", name="feat")
                for ki in range(CB):
                    nc.tensor.matmul(
                        fp[:], xc[ki][:, pb * 128:(pb + 1) * 128],
                        wkq[ki][:], start=(ki == 0), stop=False)
                nc.tensor.matmul(fp[:], ones_r[:], bias_r[:],
                                 start=False, stop=True)
                fb = fpool.tile([128, FKQ], dt, tag="fb", name="fb")
                if pb % 2 == 1:
                    nc.scalar.activation(fb[:], fp[:], AF.Copy)
                else:
                    nc.vector.tensor_copy(fb[:], fp[:])
                for cb in range(CB):
                    nc.tensor.matmul(
                        kps[cb],
                        fb[:, cb * 128:(cb + 1) * 128],
                        fb[:, C:C + TQ],
                        start=(pb == 0), stop=(pb == HW // 128 - 1))
            for cb in range(CB):
                nc.vector.tensor_copy(krnl[s][cb][:], kps[cb])

        # ---- BN stats + AllReduce ---------------------------------------
        loc = []
        for cb in range(CB):
            st = vpool.tile([128, 2], dt, tag=f"st{cb}", name=f"st{cb}")
            tmp = wpool.tile([128, TQ], dt, tag="sq", name="sq")
            prt = wpool.tile([128, 4], dt, tag="prt", name="prt")
            for s in range(NLOC):
                nc.vector.tensor_reduce(prt[:, s:s + 1], krnl[s][cb][:],
                                        AX.X, ALU.add)
                nc.vector.tensor_tensor(out=tmp[:], in0=krnl[s][cb][:],
                                        in1=krnl[s][cb][:], op=ALU.mult)
                nc.vector.tensor_reduce(prt[:, 2 + s:3 + s], tmp[:],
                                        AX.X, ALU.add)
            nc.vector.tensor_tensor(out=st[:, 0:1], in0=prt[:, 0:1],
                                    in1=prt[:, 1:2], op=ALU.add)
            nc.vector.tensor_tensor(out=st[:, 1:2], in0=prt[:, 2:3],
                                    in1=prt[:, 3:4], op=ALU.add)
            loc.append(st)

        ib = dpool.tile([CB, 128, 2], dt)
        ob = dpool.tile([CB, 128, 2], dt)
        for cb in range(CB):
            nc.gpsimd.dma_start(ib[cb], loc[cb][:])
        if os.environ.get("PROF_NO_CC"):
            nc.gpsimd.dma_start(ob[:], ib[:])
        else:
            nc.gpsimd.collective_compute(
                "AllReduce", ALU.add, replica_groups=[list(range(N_CORES))],
                ins=[ib.opt()], outs=[ob.opt()])

        eps_t = vpool.tile([128, 1], dt, tag="eps")
        nc.vector.memset(eps_t[:], BN_EPS)
        scale, shift = [], []
        for cb in range(CB):
            gl = vpool.tile([128, 2], dt, tag=f"gl{cb}", name=f"gl{cb}")
            nc.gpsimd.dma_start(gl[:], ob[cb])
            mean = wpool.tile([128, 1], dt, tag="mean", name="mean")
            sc = vpool.tile([128, 1], dt, tag=f"sc{cb}", name=f"sc{cb}")
            sh = vpool.tile([128, 1], dt, tag=f"sh{cb}", name=f"sh{cb}")
            t0 = wpool.tile([128, 1], dt, tag="bn0", name="bn0")
            t1 = wpool.tile([128, 1], dt, tag="bn1", name="bn1")
            nc.vector.tensor_scalar_mul(mean[:], gl[:, 0:1], 1.0 / BN_CNT)
            nc.vector.tensor_tensor(out=t0[:], in0=mean[:], in1=mean[:],
                                    op=ALU.mult)
            nc.vector.scalar_tensor_tensor(
                out=t1[:], in0=gl[:, 1:2], scalar=1.0 / BN_CNT, in1=t0[:],
                op0=ALU.mult, op1=ALU.subtract)
            nc.scalar.activation(t0[:], t1[:], AF.Sqrt, bias=eps_t[:])
            nc.vector.reciprocal(t1[:], t0[:])
            nc.vector.tensor_tensor(out=sc[:], in0=gam[cb][:], in1=t1[:],
                                    op=ALU.mult)
            nc.vector.tensor_tensor(out=t0[:], in0=mean[:], in1=sc[:],
                                    op=ALU.mult)
            nc.vector.tensor_tensor(out=sh[:], in0=bet[cb][:], in1=t0[:],
                                    op=ALU.subtract)
            scale.append(sc)
            shift.append(sh)

        # normalized per-tap weights: w = krnl*scale + shift
        wnorm = []
        for s in range(NLOC):
            wr = []
            for cb in range(CB):
                wn = vpool.tile([128, TQ], dt, tag=f"wn{s}{cb}",
                                name=f"wn{s}{cb}")
                nc.vector.tensor_scalar(
                    out=wn[:], in0=krnl[s][cb][:],
                    scalar1=scale[cb][:], scalar2=shift[cb][:],
                    op0=ALU.mult, op1=ALU.add)
                wr.append(wn)
            wnorm.append(wr)

        # diag tiles for all units' PE taps (bf16), generated on ScalarE
        diags = []
        for u in range(NU):
            s, cb = divmod(u, CB)
            wn = wnorm[s][cb]
            dgu = {}
            for t in DIAG_TAPS:
                dg = gpool.tile([128, 128], db, tag="diag", name=f"dg{u}_{t}")
                nc.scalar.activation(dg[:], identb[:], AF.Copy,
                                     scale=wn[:, t:t + 1])
                dgu[t] = dg
            diags.append(dgu)

        # ---- depthwise convs + sigmoid + merge --------------------------
        for u in range(NU):
            s, cb = divmod(u, CB)
            wn = wnorm[s][cb]
            # zero-padded image (fp32r; DMA rounds on load)
            pt = ppool.tile([128, PSZ], db, tag="pimg", name=f"pc{u}")
            pg = pt[:].rearrange("p (r c) -> p r c", c=WP)
            nc.gpsimd.memset(pt[:, 0:PAD * WP + PAD], 0.0)
            nc.gpsimd.memset(pt[:, PSZ - PAD * WP - PAD:PSZ], 0.0)
            nc.gpsimd.memset(pg[:, PAD:PAD + H, 0:PAD], 0.0)
            nc.gpsimd.memset(pg[:, PAD:PAD + H, PAD + W:WP], 0.0)
            for q in range(2):
                rr = H // 2
                eng = nc.sync if q == 0 else nc.scalar
                eng.dma_start(
                    pg[:, PAD + q * rr:PAD + (q + 1) * rr, PAD:PAD + W],
                    x_d.ap()[s, cb * 128:(cb + 1) * 128,
                             q * rr:(q + 1) * rr])
            diag = diags[u]
            sgs = []
            for di, d in enumerate((1, 2, 3)):
                cfg = (TAPS_LAST if u == NU - 1 else TAPS)[d]
                zb = bpool.tile([128, HW], db, tag="big", name=f"zb{u}_{di}")
                zb3 = zb[:].rearrange("p (r c) -> p r c", c=W)
                for ci in range(NCHUNK):
                    r0 = ci * CH_ROWS
                    pz = psA.tile([128, CSPAN], dt, tag="ps", name=f"pz{di}")
                    for i, t in enumerate(cfg["pe"]):
                        dy, dx = tap_dydx(t, d)
                        nc.tensor.matmul(
                            pz[:], diag[t][:],
                            pwin(pt, r0, CH_ROWS, dy, dx),
                            start=(i == 0),
                            stop=(i == len(cfg["pe"]) - 1))
                    # fused center tap + psum evacuation
                    nc.vector.scalar_tensor_tensor(
                        out=zb3[:, r0:r0 + CH_ROWS, :],
                        in0=pwin(pt, r0, CH_ROWS, 0, 0),
                        scalar=wn[:, 4:5],
                        in1=pz[:].rearrange("p (r c) -> p r c", c=W),
                        op0=ALU.mult, op1=ALU.add)
                for t in cfg["dve"]:
                    dy, dx = tap_dydx(t, d)
                    nc.vector.scalar_tensor_tensor(
                        out=zb3, in0=pwin(pt, 0, H, dy, dx),
                        scalar=wn[:, t:t + 1], in1=zb3,
                        op0=ALU.mult, op1=ALU.add)
                for t in cfg["gp"]:
                    dy, dx = tap_dydx(t, d)
                    ga = apool.tile([128, HW], db, tag="ga",
                                    name=f"ga{u}_{di}")
                    nc.gpsimd.tensor_scalar_mul(
                        ga[:].rearrange("p (r c) -> p r c", c=W),
                        pwin(pt, 0, H, dy, dx), wn[:, t:t + 1])
                    nc.gpsimd.tensor_tensor(out=zb[:], in0=zb[:], in1=ga[:],
                                            op=ALU.add)
                sg = spool.tile([128, HW], db, tag="sig", name=f"sg{u}_{di}")
                nc.scalar.activation(sg[:], zb[:], AF.Sigmoid)
                sgs.append(sg)
            ext = spool.tile([128, HW], db, tag="sig", name=f"ext{u}")
            for off in range(0, HW, MG):
                pm = psB.tile([128, MG], dt, tag="mg", name=f"pm{u}")
                for di in range(3):
                    nc.tensor.matmul(pm[:], identb[:],
                                     sgs[di][:, off:off + MG],
                                     start=(di == 0), stop=(di == 2))
                nc.scalar.activation(ext[:, off:off + MG], pm[:], AF.Copy,
                                     scale=1.0 / 3.0)
            for q in range(2):
                nc.sync.dma_start(
                    out_d.ap()[s, cb * 128:(cb + 1) * 128,
                               q * 32:(q + 1) * 32],
                    ext[:].rearrange("p (r c) -> p r c", c=W)[
                        :, q * 32:(q + 1) * 32, :])


def _build():
    nc = bacc.Bacc("TRN2", debug=False, num_devices=N_CORES,
                   target_bir_lowering=False)
    x_d = nc.dram_tensor("x", [NLOC, C, H, W], db, kind="ExternalInput")
    w_d = nc.dram_tensor("wkqt", [C, FKQ], db, kind="ExternalInput")
    bk_d = nc.dram_tensor("bk", [C], dt, kind="ExternalInput")
    bq_d = nc.dram_tensor("bq", [FQ], dt, kind="ExternalInput")
    g_d = nc.dram_tensor("gamma", [C], dt, kind="ExternalInput")
    b_d = nc.dram_tensor("beta", [C], dt, kind="ExternalInput")
    out_d = nc.dram_tensor("out", [NLOC, C, H, W], db, kind="ExternalOutput")
    with tile.TileContext(nc) as tc:
        _body(nc, tc, (x_d, w_d, bk_d, bq_d, g_d, b_d, out_d))
    nc.compile()
    return nc


_nc_cache = None
last_results = None


def kernel(x, wk, bk, wq, bq, gamma, beta):
    global _nc_cache, last_results
    if _nc_cache is None:
        _nc_cache = _build()
    nc = _nc_cache
    x = np.ascontiguousarray(x, dtype=np.float32).astype(ml_dtypes.bfloat16)
    wkqt = np.concatenate(
        [np.asarray(wk, np.float32).T, np.asarray(wq, np.float32).T,
         np.zeros((C, TQ - FQ), np.float32)], axis=1)  # [C, 266]
    wkqt = np.ascontiguousarray(wkqt.astype(ml_dtypes.bfloat16))
    in_maps = []
    for c in range(N_CORES):
        sl = slice(c * NLOC, (c + 1) * NLOC)
        in_maps.append({
            "x": x[sl],
            "wkqt": wkqt,
            "bk": np.ascontiguousarray(bk, np.float32),
            "bq": np.ascontiguousarray(bq, np.float32),
            "gamma": np.ascontiguousarray(gamma, np.float32),
            "beta": np.ascontiguousarray(beta, np.float32),
        })
    res = bass_utils.run_bass_kernel_spmd(
        nc, in_maps, core_ids=list(range(N_CORES)))
    last_results = res
    out = np.concatenate([res.results[c]["out"] for c in range(N_CORES)],
                         axis=0)
    return out.astype(np.float32)
